# revision 13
# baseline (speedup 1.0000x reference)
"""Position-attention kernel for Trainium2 (8 NeuronCores, SPMD data-parallel).

Math (per batch b):
    q = X Wq ; k = X Wk ; v = X Wv          (X = x[b] reshaped [N, C], N=4096, C=128)
    energy[i, j] = k_i . q_j
    attn = softmax(energy, axis=-1)
    out = gamma * (attn @ v) + X

Kernel restructuring:
    energy = X A X^T with A = Wq Wk^T, computed transposed as
    eT[j, i] = sum_c xT[c, j] * w[c, i].  The small factors w = A X_i^T
    ([128, 2048]) and v = X Wv are precomputed on the host (like A itself)
    and streamed in, so the device pipeline is pure energy -> exp -> attn@v.
    eT lands in PSUM with j on partitions and is exponentiated directly into
    SBUF as bf16 -> already in the right layout to be the stationary operand
    of the attn@v matmul (no transposes anywhere). A ones-column appended to
    v gives the softmax denominator for free.

    The exp is SPLIT between two engines per group:
      - ACT (scalar): exact exp on i-columns [0:SPLIT) of each 256-wide chunk.
      - DVE (vector): Schraudolph-style exp on columns [SPLIT:256): one
        tensor_scalar computes round(e*SA + SB) with a saturating f32->uint16
        conversion (negatives clamp to 0 = exp underflow; verified on HW),
        whose bits ARE the bf16 representation of exp(e-SHIFT)*2^(-sigma/128).
        The sigma bias cancels in softmax normalization; measured end-to-end
        error of this approximation is ~2e-3 (tolerance 2e-2).

Sharding: 8 cores = (4 batches) x (2 halves of the 4096 output rows).
"""

import math

import numpy as np

B, Dd, Hh, Ww, C = 4, 16, 16, 16, 128
N = Dd * Hh * Ww            # 4096 sequence positions (j)
NCORES = 8
NI = (B * N) // NCORES      # 2048 output rows per core (i)
NJB = N // 128              # 32 j-blocks
G = 4                       # j-blocks per exp group (PSUM: 3*2 + 2 banks = 8)
IC = 256                    # i-chunk (2 accumulator tiles of 128 rows)
NICH = NI // IC             # 8 i-chunks
NIT = IC // 128             # 2 i-tiles per chunk
SHIFT = 32.0                # softmax shift (cancels exactly in normalization)
SPLIT = 160                 # i-cols [0:SPLIT) exp'd by ACT, [SPLIT:256) by DVE

# Schraudolph constants: bf16 bits = round(e * SA + SB), saturating to [0, 65535]
SA = 128.0 / math.log(2.0)
SB = 127.0 * 128.0 - SHIFT * SA - 5.5

_NC_CACHE = {}


def _build_nc():
    from contextlib import ExitStack

    import concourse.bacc as bacc
    import concourse.bass as bass
    import concourse.mybir as mybir
    import concourse.tile as tile

    dt = mybir.dt
    nc = bacc.Bacc(target_bir_lowering=False)

    xT_d = nc.declare_dram_parameter("xT", [128, N], dt.float16, isOutput=False)
    w_d = nc.declare_dram_parameter("w", [128, NI], dt.float16, isOutput=False)
    v_d = nc.declare_dram_parameter(
        "v", [N // 512, 128, 4, 128], dt.bfloat16, isOutput=False
    )
    xres_d = nc.declare_dram_parameter(
        "xres", [NI // 128, 128, 128], dt.float32, isOutput=False
    )
    gam_d = nc.declare_dram_parameter("gam", [1, 1], dt.float32, isOutput=False)
    out_d = nc.declare_dram_parameter(
        "out", [NI // 128, 128, 128], dt.float32, isOutput=True
    )

    NCH = N // 512   # 8 column chunks of xT / row chunks of v
    with tile.TileContext(nc) as tc, ExitStack() as ctx:
        persist = ctx.enter_context(tc.tile_pool(name="persist", bufs=1))

        # warm up the exp table load while DMAs run
        dummy = persist.tile([1, 1], dt.float32)
        nc.vector.memset(dummy[:], 0.0)
        nc.scalar.activation(
            out=dummy[:], in_=dummy[:], func=mybir.ActivationFunctionType.Exp
        )
        # zeroed operand for PE-warmup matmuls
        warm = persist.tile([128, 128], dt.float16)
        nc.vector.memset(warm[:], 0.0)

        # tiny queue-warmers absorb DMA-ring init latency, then DMA issue
        # order mirrors first use: xt0+w0 feed the first energy group, v0
        # the first attn@v; later chunks alternate between the two queues
        qw = persist.tile([1, 4], dt.float16)
        nc.sync.dma_start(out=qw[0:1, 0:2], in_=xT_d[0:1, 0:2])
        nc.gpsimd.dma_start(out=qw[0:1, 2:4], in_=xT_d[0:1, 2:4])

        xt_ch = [
            persist.tile([128, 512], dt.float16, name=f"xt{jc}") for jc in range(NCH)
        ]
        v_ch = [
            persist.tile([128, 4, 132], dt.bfloat16, name=f"v{jc}")
            for jc in range(NCH)
        ]
        w_t = persist.tile([128, NI], dt.float16)

        def ld_xt(eng, jc):
            eng.dma_start(out=xt_ch[jc][:], in_=xT_d[:, jc * 512 : (jc + 1) * 512])

        def ld_v(eng, jc):
            eng.dma_start(out=v_ch[jc][:, :, 0:128], in_=v_d[jc])

        def ld_w(eng, k):
            eng.dma_start(
                out=w_t[:, k * 512 : (k + 1) * 512],
                in_=w_d[:, k * 512 : (k + 1) * 512],
            )

        nc.sync.dma_start(out=w_t[:, 0:256], in_=w_d[:, 0:256])
        nc.sync.dma_start(out=xt_ch[0][:, 0:256], in_=xT_d[:, 0:256])
        nc.sync.dma_start(out=xt_ch[0][:, 256:512], in_=xT_d[:, 256:512])
        # interleaved by time of first use (energy g needs xt g; attn@v g
        # needs v g one exp later; w columns 256: only at i-chunk 1+)
        ld_xt(nc.gpsimd, 1)
        ld_v(nc.gpsimd, 0)
        ld_xt(nc.sync, 2)
        ld_v(nc.gpsimd, 1)
        ld_xt(nc.sync, 3)
        ld_v(nc.gpsimd, 2)
        ld_xt(nc.sync, 4)
        ld_v(nc.gpsimd, 3)
        ld_xt(nc.sync, 5)
        ld_v(nc.gpsimd, 4)
        ld_xt(nc.sync, 6)
        ld_v(nc.gpsimd, 5)
        ld_xt(nc.sync, 7)
        ld_v(nc.gpsimd, 6)
        ld_v(nc.sync, 7)
        nc.gpsimd.dma_start(out=w_t[:, 256:512], in_=w_d[:, 256:512])
        ld_w(nc.gpsimd, 1)
        ld_w(nc.sync, 2)
        ld_w(nc.gpsimd, 3)
        gam = persist.tile([128, 1], dt.float32)
        gam_ap = gam_d[:, :]
        nc.gpsimd.dma_start(
            out=gam[:],
            in_=bass.AP(
                tensor=gam_ap.tensor, offset=gam_ap.offset, ap=[[0, 128], [1, 1]]
            ),
        )
        shiftb = persist.tile([128, 1], dt.float32)
        nc.vector.memset(shiftb[:], -SHIFT)
        for jc in range(NCH):
            nc.vector.memset(v_ch[jc][:, :, 128:129], 1.0)

        epool = ctx.enter_context(tc.tile_pool(name="epsum", bufs=3, space="PSUM"))
        opool = ctx.enter_context(tc.tile_pool(name="opsum", bufs=1, space="PSUM"))
        ptpool = ctx.enter_context(tc.tile_pool(name="ptp", bufs=8))
        spool = ctx.enter_context(tc.tile_pool(name="small", bufs=8))
        xrpool = ctx.enter_context(tc.tile_pool(name="xrp", bufs=3))
        outpool = ctx.enter_context(tc.tile_pool(name="outp", bufs=3))

        # ---- PE warmup (also spins the pstate clock up) ----
        wt = opool.tile([128, 512], dt.float32, tag="oa0", name="warmp")
        for r in range(8):
            nc.tensor.matmul(
                wt[:, r * 64 : (r + 1) * 64],
                warm[:],
                warm[:, 0:64],
                start=True,
                stop=True,
            )

        # ---- main loop ----
        ngroups = NJB // G

        def emit_energy(icn, gi):
            et = epool.tile([128, G, IC], dt.float32, tag="et", name=f"et{icn}_{gi}")
            wsl = w_t[:, icn * IC : (icn + 1) * IC]
            for g in range(G):
                jb = gi * G + g
                nc.tensor.matmul(
                    et[:, g, :],
                    xt_ch[jb // 4][:, (jb % 4) * 128 : (jb % 4 + 1) * 128],
                    wsl,
                    start=True,
                    stop=True,
                )
            return et

        def emit_exp(icn, gi):
            et = ets.pop((icn, gi))
            pt = ptpool.tile(
                [128, G, IC], dt.bfloat16, tag="pt", name=f"pt{icn}_{gi}"
            )
            # DVE: Schraudolph exp via affine + saturating u16 convert
            nc.vector.tensor_scalar(
                out=pt[:, :, SPLIT:IC].bitcast(dt.uint16),
                in0=et[:, :, SPLIT:IC],
                scalar1=float(SA),
                scalar2=float(SB),
                op0=mybir.AluOpType.mult,
                op1=mybir.AluOpType.add,
            )
            # ACT: exact exp
            nc.scalar.activation(
                out=pt[:, :, 0:SPLIT],
                in_=et[:, :, 0:SPLIT],
                func=mybir.ActivationFunctionType.Exp,
                bias=shiftb[:],
            )
            return pt

        def emit_attnv(icn, gi, pt):
            oat = oa_by_ic[icn]
            for g in range(G):
                jb = gi * G + g
                for it in range(NIT):
                    # start=True clears has_written for the WHOLE bank, so only
                    # it=0 may set it; it=1's first write lands on cleared bits
                    # and therefore overwrites (= start) without re-clearing.
                    nc.tensor.matmul(
                        oat[:, it, :],
                        pt[:, g, it * 128 : (it + 1) * 128],
                        v_ch[jb // 4][:, jb % 4, 0:129],
                        start=(jb == 0 and it == 0),
                        stop=(jb == NJB - 1),
                        skip_group_check=(jb == 0 and it == 1),
                    )

        def alloc_oa(icn):
            # ping-pong by chunk parity so attn@v of chunk n+1 accumulates
            # while blend of chunk n still reads the other bank
            oa_by_ic[icn] = opool.tile(
                [128, NIT, 129], dt.float32, tag=f"oa{icn % 2}", name=f"oat{icn}"
            )

        def emit_blend(icn):
            oat = oa_by_ic.pop(icn)
            for it in range(NIT):
                ti = icn * NIT + it
                rs = spool.tile([128, 1], dt.float32, tag="rs", name=f"rs{ti}")
                nc.vector.reciprocal(rs[:], oat[:, it, 128:129])
                nc.vector.tensor_scalar(
                    out=rs[:],
                    in0=rs[:],
                    scalar1=gam[:],
                    scalar2=None,
                    op0=mybir.AluOpType.mult,
                )
                xr = xrpool.tile([128, 128], dt.float32, tag="xr", name=f"xr{ti}")
                nc.sync.dma_start(out=xr[:], in_=xres_d[ti])
                ot = outpool.tile([128, 128], dt.float32, tag="ot", name=f"ot{ti}")
                # fused: out = (attn_num * gamma/den) + x_residual
                nc.vector.scalar_tensor_tensor(
                    out=ot[:],
                    in0=oat[:, it, 0:128],
                    scalar=rs[:],
                    in1=xr[:],
                    op0=mybir.AluOpType.mult,
                    op1=mybir.AluOpType.add,
                )
                nc.gpsimd.dma_start(out=out_d[ti], in_=ot[:])

        ets = {}
        oa_by_ic = {}

        # uniform flat schedule over all (i-chunk, group) pairs with 3-group
        # PE lookahead (3 et slots). The lookahead energy is emitted AFTER
        # this group's attn@v so a slot-blocked energy never head-of-line-
        # blocks ready attn@v work on the PE queue. blend(n) is emitted one
        # group into chunk n+1 so its vector ops don't delay the next chunk's
        # exp on the DVE queue.
        flat = [(icn, gi) for icn in range(NICH) for gi in range(ngroups)]
        for k in range(3):
            ets[flat[k]] = emit_energy(*flat[k])
        for fk, (icn, gi) in enumerate(flat):
            pt = emit_exp(icn, gi)
            if gi == 0:
                alloc_oa(icn)
            emit_attnv(icn, gi, pt)
            if fk + 3 < len(flat):
                ets[flat[fk + 3]] = emit_energy(*flat[fk + 3])
            if gi == 1 and icn > 0:
                emit_blend(icn - 1)
        emit_blend(NICH - 1)

    nc.finalize()
    return nc


def get_nc():
    if "nc" not in _NC_CACHE:
        _NC_CACHE["nc"] = _build_nc()
    return _NC_CACHE["nc"]


def make_in_maps(x, Wq, Wk, Wv, gamma):
    import ml_dtypes

    x = np.asarray(x, dtype=np.float32)
    Wq = np.asarray(Wq, dtype=np.float32)
    Wk = np.asarray(Wk, dtype=np.float32)
    Wv = np.asarray(Wv, dtype=np.float32)
    gamma = np.asarray(gamma, dtype=np.float32)

    xf = x.reshape(B, N, C)
    A = Wq @ Wk.T
    gam = gamma.reshape(1, 1)

    in_maps = []
    for c in range(NCORES):
        b, ih = c // 2, c % 2
        xT = xf[b].T  # [128, 4096]
        # rotate the j-order so this core's own i-rows are columns 0:NI
        # (softmax sums over j, so any j-order works as long as v matches)
        xTr = np.ascontiguousarray(np.roll(xT, -ih * NI, axis=1)).astype(np.float16)
        sl = slice(ih * NI, (ih + 1) * NI)
        # w[c, i] = (A @ x_i)_c for this core's i rows (host prep, fp32->fp16)
        w = (A @ xf[b][sl].T).astype(np.float16)
        # v rows follow the same rotated j-order; laid out [jc, p, k, c] to
        # match the [128p, 4k, 128c] SBUF tiles
        v = np.roll(xf[b] @ Wv, -ih * NI, axis=0)
        v = np.ascontiguousarray(
            v.reshape(NCH_V, 4, 128, C).transpose(0, 2, 1, 3)
        ).astype(ml_dtypes.bfloat16)
        in_maps.append(
            {
                "xT": xTr,
                "w": np.ascontiguousarray(w),
                "v": v,
                "xres": np.ascontiguousarray(
                    xf[b][sl].reshape(NI // 128, 128, 128)
                ),
                "gam": gam,
            }
        )
    return in_maps


NCH_V = N // 512


def assemble_out(results):
    outs = [np.asarray(results[c]["out"]).reshape(NI, C) for c in range(NCORES)]
    full = np.stack(
        [np.concatenate([outs[2 * b], outs[2 * b + 1]], axis=0) for b in range(B)]
    )
    return full.reshape(B, Dd, Hh, Ww, C).astype(np.float32)


def kernel(x, Wq, Wk, Wv, gamma):
    from concourse.bass_utils import run_bass_kernel_spmd

    nc = get_nc()
    in_maps = make_in_maps(x, Wq, Wk, Wv, gamma)
    res = run_bass_kernel_spmd(nc, in_maps, core_ids=list(range(NCORES)))
    return assemble_out(res.results)


# revision 14
# speedup vs baseline: 1.0005x; 1.0005x over previous
"""Position-attention kernel for Trainium2 (8 NeuronCores, SPMD data-parallel).

Math (per batch b):
    q = X Wq ; k = X Wk ; v = X Wv          (X = x[b] reshaped [N, C], N=4096, C=128)
    energy[i, j] = k_i . q_j
    attn = softmax(energy, axis=-1)
    out = gamma * (attn @ v) + X

Kernel restructuring:
    energy = X A X^T with A = Wq Wk^T, computed transposed as
    eT[j, i] = sum_c xT[c, j] * w[c, i].  The small factors w = A X_i^T
    ([128, 2048]) and v = X Wv are precomputed on the host (like A itself)
    and streamed in, so the device pipeline is pure energy -> exp -> attn@v.
    eT lands in PSUM with j on partitions and is exponentiated directly into
    SBUF as bf16 -> already in the right layout to be the stationary operand
    of the attn@v matmul (no transposes anywhere). A ones-column appended to
    v gives the softmax denominator for free.

    The exp is SPLIT between two engines per group:
      - ACT (scalar): exact exp on i-columns [0:SPLIT) of each 256-wide chunk.
      - DVE (vector): Schraudolph-style exp on columns [SPLIT:256): one
        tensor_scalar computes round(e*SA + SB) with a saturating f32->uint16
        conversion (negatives clamp to 0 = exp underflow; verified on HW),
        whose bits ARE the bf16 representation of exp(e-SHIFT)*2^(-sigma/128).
        The sigma bias cancels in softmax normalization; measured end-to-end
        error of this approximation is ~2e-3 (tolerance 2e-2).

Sharding: 8 cores = (4 batches) x (2 halves of the 4096 output rows).
"""

import math

import numpy as np

B, Dd, Hh, Ww, C = 4, 16, 16, 16, 128
N = Dd * Hh * Ww            # 4096 sequence positions (j)
NCORES = 8
NI = (B * N) // NCORES      # 2048 output rows per core (i)
NJB = N // 128              # 32 j-blocks
G = 4                       # j-blocks per exp group (PSUM: 3*2 + 2 banks = 8)
IC = 256                    # i-chunk (2 accumulator tiles of 128 rows)
NICH = NI // IC             # 8 i-chunks
NIT = IC // 128             # 2 i-tiles per chunk
SHIFT = 32.0                # softmax shift (cancels exactly in normalization)
SPLIT = 160                 # i-cols [0:SPLIT) exp'd by ACT, [SPLIT:256) by DVE

# Schraudolph constants: bf16 bits = round(e * SA + SB), saturating to [0, 65535]
SA = 128.0 / math.log(2.0)
SB = 127.0 * 128.0 - SHIFT * SA - 5.5

_NC_CACHE = {}


def _build_nc():
    from contextlib import ExitStack

    import concourse.bacc as bacc
    import concourse.bass as bass
    import concourse.mybir as mybir
    import concourse.tile as tile

    dt = mybir.dt
    nc = bacc.Bacc(target_bir_lowering=False)

    xT_d = nc.declare_dram_parameter("xT", [128, N], dt.float16, isOutput=False)
    w_d = nc.declare_dram_parameter("w", [128, NI], dt.float16, isOutput=False)
    v_d = nc.declare_dram_parameter(
        "v", [N // 512, 128, 4, 128], dt.bfloat16, isOutput=False
    )
    xres_d = nc.declare_dram_parameter(
        "xres", [NI // 128, 128, 128], dt.float32, isOutput=False
    )
    gam_d = nc.declare_dram_parameter("gam", [1, 1], dt.float32, isOutput=False)
    out_d = nc.declare_dram_parameter(
        "out", [NI // 128, 128, 128], dt.float32, isOutput=True
    )

    NCH = N // 512   # 8 column chunks of xT / row chunks of v
    with tile.TileContext(nc) as tc, ExitStack() as ctx:
        persist = ctx.enter_context(tc.tile_pool(name="persist", bufs=1))

        # warm up the exp table load while DMAs run
        dummy = persist.tile([1, 1], dt.float32)
        nc.vector.memset(dummy[:], 0.0)
        nc.scalar.activation(
            out=dummy[:], in_=dummy[:], func=mybir.ActivationFunctionType.Exp
        )
        # zeroed operand for PE-warmup matmuls
        warm = persist.tile([128, 128], dt.float16)
        nc.vector.memset(warm[:], 0.0)

        # tiny queue-warmers absorb DMA-ring init latency, then DMA issue
        # order mirrors first use: xt0+w0 feed the first energy group, v0
        # the first attn@v; later chunks alternate between the two queues
        qw = persist.tile([1, 4], dt.float16)
        nc.sync.dma_start(out=qw[0:1, 0:2], in_=xT_d[0:1, 0:2])
        nc.gpsimd.dma_start(out=qw[0:1, 2:4], in_=xT_d[0:1, 2:4])

        xt_ch = [
            persist.tile([128, 512], dt.float16, name=f"xt{jc}") for jc in range(NCH)
        ]
        v_ch = [
            persist.tile([128, 4, 132], dt.bfloat16, name=f"v{jc}")
            for jc in range(NCH)
        ]
        w_t = persist.tile([128, NI], dt.float16)

        def ld_xt(eng, jc):
            eng.dma_start(out=xt_ch[jc][:], in_=xT_d[:, jc * 512 : (jc + 1) * 512])

        def ld_v(eng, jc):
            eng.dma_start(out=v_ch[jc][:, :, 0:128], in_=v_d[jc])

        def ld_w(eng, k):
            eng.dma_start(
                out=w_t[:, k * 512 : (k + 1) * 512],
                in_=w_d[:, k * 512 : (k + 1) * 512],
            )

        nc.sync.dma_start(out=w_t[:, 0:256], in_=w_d[:, 0:256])
        nc.sync.dma_start(out=xt_ch[0][:, 0:256], in_=xT_d[:, 0:256])
        nc.scalar.dma_start(out=xt_ch[0][:, 256:512], in_=xT_d[:, 256:512])
        # interleaved by time of first use (energy g needs xt g; attn@v g
        # needs v g one exp later; w columns 256: only at i-chunk 1+)
        ld_xt(nc.gpsimd, 1)
        ld_v(nc.gpsimd, 0)
        ld_xt(nc.sync, 2)
        ld_v(nc.gpsimd, 1)
        ld_xt(nc.sync, 3)
        ld_v(nc.gpsimd, 2)
        ld_xt(nc.sync, 4)
        ld_v(nc.gpsimd, 3)
        ld_xt(nc.sync, 5)
        ld_v(nc.gpsimd, 4)
        ld_xt(nc.sync, 6)
        ld_v(nc.gpsimd, 5)
        ld_xt(nc.sync, 7)
        ld_v(nc.gpsimd, 6)
        ld_v(nc.sync, 7)
        nc.gpsimd.dma_start(out=w_t[:, 256:512], in_=w_d[:, 256:512])
        ld_w(nc.gpsimd, 1)
        ld_w(nc.sync, 2)
        ld_w(nc.gpsimd, 3)
        gam = persist.tile([128, 1], dt.float32)
        gam_ap = gam_d[:, :]
        nc.gpsimd.dma_start(
            out=gam[:],
            in_=bass.AP(
                tensor=gam_ap.tensor, offset=gam_ap.offset, ap=[[0, 128], [1, 1]]
            ),
        )
        shiftb = persist.tile([128, 1], dt.float32)
        nc.vector.memset(shiftb[:], -SHIFT)
        for jc in range(NCH):
            nc.vector.memset(v_ch[jc][:, :, 128:129], 1.0)

        epool = ctx.enter_context(tc.tile_pool(name="epsum", bufs=3, space="PSUM"))
        opool = ctx.enter_context(tc.tile_pool(name="opsum", bufs=1, space="PSUM"))
        ptpool = ctx.enter_context(tc.tile_pool(name="ptp", bufs=8))
        spool = ctx.enter_context(tc.tile_pool(name="small", bufs=8))
        xrpool = ctx.enter_context(tc.tile_pool(name="xrp", bufs=3))
        outpool = ctx.enter_context(tc.tile_pool(name="outp", bufs=3))

        # ---- PE warmup (also spins the pstate clock up) ----
        wt = opool.tile([128, 512], dt.float32, tag="oa0", name="warmp")
        for r in range(8):
            nc.tensor.matmul(
                wt[:, r * 64 : (r + 1) * 64],
                warm[:],
                warm[:, 0:64],
                start=True,
                stop=True,
            )

        # ---- main loop ----
        ngroups = NJB // G

        def emit_energy(icn, gi):
            et = epool.tile([128, G, IC], dt.float32, tag="et", name=f"et{icn}_{gi}")
            wsl = w_t[:, icn * IC : (icn + 1) * IC]
            for g in range(G):
                jb = gi * G + g
                nc.tensor.matmul(
                    et[:, g, :],
                    xt_ch[jb // 4][:, (jb % 4) * 128 : (jb % 4 + 1) * 128],
                    wsl,
                    start=True,
                    stop=True,
                )
            return et

        def emit_exp(icn, gi):
            et = ets.pop((icn, gi))
            pt = ptpool.tile(
                [128, G, IC], dt.bfloat16, tag="pt", name=f"pt{icn}_{gi}"
            )
            # DVE: Schraudolph exp via affine + saturating u16 convert
            nc.vector.tensor_scalar(
                out=pt[:, :, SPLIT:IC].bitcast(dt.uint16),
                in0=et[:, :, SPLIT:IC],
                scalar1=float(SA),
                scalar2=float(SB),
                op0=mybir.AluOpType.mult,
                op1=mybir.AluOpType.add,
            )
            # ACT: exact exp
            nc.scalar.activation(
                out=pt[:, :, 0:SPLIT],
                in_=et[:, :, 0:SPLIT],
                func=mybir.ActivationFunctionType.Exp,
                bias=shiftb[:],
            )
            return pt

        def emit_attnv(icn, gi, pt):
            oat = oa_by_ic[icn]
            for g in range(G):
                jb = gi * G + g
                for it in range(NIT):
                    # start=True clears has_written for the WHOLE bank, so only
                    # it=0 may set it; it=1's first write lands on cleared bits
                    # and therefore overwrites (= start) without re-clearing.
                    nc.tensor.matmul(
                        oat[:, it, :],
                        pt[:, g, it * 128 : (it + 1) * 128],
                        v_ch[jb // 4][:, jb % 4, 0:129],
                        start=(jb == 0 and it == 0),
                        stop=(jb == NJB - 1),
                        skip_group_check=(jb == 0 and it == 1),
                    )

        def alloc_oa(icn):
            # ping-pong by chunk parity so attn@v of chunk n+1 accumulates
            # while blend of chunk n still reads the other bank
            oa_by_ic[icn] = opool.tile(
                [128, NIT, 129], dt.float32, tag=f"oa{icn % 2}", name=f"oat{icn}"
            )

        def emit_blend(icn):
            oat = oa_by_ic.pop(icn)
            for it in range(NIT):
                ti = icn * NIT + it
                rs = spool.tile([128, 1], dt.float32, tag="rs", name=f"rs{ti}")
                nc.vector.reciprocal(rs[:], oat[:, it, 128:129])
                nc.vector.tensor_scalar(
                    out=rs[:],
                    in0=rs[:],
                    scalar1=gam[:],
                    scalar2=None,
                    op0=mybir.AluOpType.mult,
                )
                xr = xrpool.tile([128, 128], dt.float32, tag="xr", name=f"xr{ti}")
                nc.sync.dma_start(out=xr[:], in_=xres_d[ti])
                ot = outpool.tile([128, 128], dt.float32, tag="ot", name=f"ot{ti}")
                # fused: out = (attn_num * gamma/den) + x_residual
                nc.vector.scalar_tensor_tensor(
                    out=ot[:],
                    in0=oat[:, it, 0:128],
                    scalar=rs[:],
                    in1=xr[:],
                    op0=mybir.AluOpType.mult,
                    op1=mybir.AluOpType.add,
                )
                nc.gpsimd.dma_start(out=out_d[ti], in_=ot[:])

        ets = {}
        oa_by_ic = {}

        # uniform flat schedule over all (i-chunk, group) pairs with 3-group
        # PE lookahead (3 et slots). The lookahead energy is emitted AFTER
        # this group's attn@v so a slot-blocked energy never head-of-line-
        # blocks ready attn@v work on the PE queue. blend(n) is emitted one
        # group into chunk n+1 so its vector ops don't delay the next chunk's
        # exp on the DVE queue.
        flat = [(icn, gi) for icn in range(NICH) for gi in range(ngroups)]
        for k in range(3):
            ets[flat[k]] = emit_energy(*flat[k])
        for fk, (icn, gi) in enumerate(flat):
            pt = emit_exp(icn, gi)
            if gi == 0:
                alloc_oa(icn)
            emit_attnv(icn, gi, pt)
            if fk + 3 < len(flat):
                ets[flat[fk + 3]] = emit_energy(*flat[fk + 3])
            if gi == 1 and icn > 0:
                emit_blend(icn - 1)
        emit_blend(NICH - 1)

    nc.finalize()
    return nc


def get_nc():
    if "nc" not in _NC_CACHE:
        _NC_CACHE["nc"] = _build_nc()
    return _NC_CACHE["nc"]


def make_in_maps(x, Wq, Wk, Wv, gamma):
    import ml_dtypes

    x = np.asarray(x, dtype=np.float32)
    Wq = np.asarray(Wq, dtype=np.float32)
    Wk = np.asarray(Wk, dtype=np.float32)
    Wv = np.asarray(Wv, dtype=np.float32)
    gamma = np.asarray(gamma, dtype=np.float32)

    xf = x.reshape(B, N, C)
    A = Wq @ Wk.T
    gam = gamma.reshape(1, 1)

    in_maps = []
    for c in range(NCORES):
        b, ih = c // 2, c % 2
        xT = xf[b].T  # [128, 4096]
        # rotate the j-order so this core's own i-rows are columns 0:NI
        # (softmax sums over j, so any j-order works as long as v matches)
        xTr = np.ascontiguousarray(np.roll(xT, -ih * NI, axis=1)).astype(np.float16)
        sl = slice(ih * NI, (ih + 1) * NI)
        # w[c, i] = (A @ x_i)_c for this core's i rows (host prep, fp32->fp16)
        w = (A @ xf[b][sl].T).astype(np.float16)
        # v rows follow the same rotated j-order; laid out [jc, p, k, c] to
        # match the [128p, 4k, 128c] SBUF tiles
        v = np.roll(xf[b] @ Wv, -ih * NI, axis=0)
        v = np.ascontiguousarray(
            v.reshape(NCH_V, 4, 128, C).transpose(0, 2, 1, 3)
        ).astype(ml_dtypes.bfloat16)
        in_maps.append(
            {
                "xT": xTr,
                "w": np.ascontiguousarray(w),
                "v": v,
                "xres": np.ascontiguousarray(
                    xf[b][sl].reshape(NI // 128, 128, 128)
                ),
                "gam": gam,
            }
        )
    return in_maps


NCH_V = N // 512


def assemble_out(results):
    outs = [np.asarray(results[c]["out"]).reshape(NI, C) for c in range(NCORES)]
    full = np.stack(
        [np.concatenate([outs[2 * b], outs[2 * b + 1]], axis=0) for b in range(B)]
    )
    return full.reshape(B, Dd, Hh, Ww, C).astype(np.float32)


def kernel(x, Wq, Wk, Wv, gamma):
    from concourse.bass_utils import run_bass_kernel_spmd

    nc = get_nc()
    in_maps = make_in_maps(x, Wq, Wk, Wv, gamma)
    res = run_bass_kernel_spmd(nc, in_maps, core_ids=list(range(NCORES)))
    return assemble_out(res.results)


# revision 16
# speedup vs baseline: 1.0100x; 1.0095x over previous
"""Position-attention kernel for Trainium2 (8 NeuronCores, SPMD data-parallel).

Math (per batch b):
    q = X Wq ; k = X Wk ; v = X Wv          (X = x[b] reshaped [N, C], N=4096, C=128)
    energy[i, j] = k_i . q_j
    attn = softmax(energy, axis=-1)
    out = gamma * (attn @ v) + X

Kernel restructuring:
    energy = X A X^T with A = Wq Wk^T, computed transposed as
    eT[j, i] = sum_c xT[c, j] * w[c, i].  The small factors w = A X_i^T
    ([128, 2048]) and v = X Wv are precomputed on the host (like A itself)
    and streamed in, so the device pipeline is pure energy -> exp -> attn@v.
    eT lands in PSUM with j on partitions and is exponentiated directly into
    SBUF as bf16 -> already in the right layout to be the stationary operand
    of the attn@v matmul (no transposes anywhere). A ones-column appended to
    v gives the softmax denominator for free.

    The exp is SPLIT between two engines per group:
      - ACT (scalar): exact exp on i-columns [0:SPLIT) of each 256-wide chunk.
      - DVE (vector): Schraudolph-style exp on columns [SPLIT:256): one
        tensor_scalar computes round(e*SA + SB) with a saturating f32->uint16
        conversion (negatives clamp to 0 = exp underflow; verified on HW),
        whose bits ARE the bf16 representation of exp(e-SHIFT)*2^(-sigma/128).
        The sigma bias cancels in softmax normalization; measured end-to-end
        error of this approximation is ~2e-3 (tolerance 2e-2).

Sharding: 8 cores = (4 batches) x (2 halves of the 4096 output rows).
"""

import math

import numpy as np

B, Dd, Hh, Ww, C = 4, 16, 16, 16, 128
N = Dd * Hh * Ww            # 4096 sequence positions (j)
NCORES = 8
NI = (B * N) // NCORES      # 2048 output rows per core (i)
NJB = N // 128              # 32 j-blocks
G = 4                       # j-blocks per exp group (PSUM: 3*2 + 2 banks = 8)
IC = 256                    # i-chunk (2 accumulator tiles of 128 rows)
NICH = NI // IC             # 8 i-chunks
NIT = IC // 128             # 2 i-tiles per chunk
SHIFT = 32.0                # softmax shift (cancels exactly in normalization)
SPLIT = 160                 # i-cols [0:SPLIT) exp'd by ACT, [SPLIT:256) by DVE

# Schraudolph constants: bf16 bits = round(e * SA + SB), saturating to [0, 65535]
SA = 128.0 / math.log(2.0)
SB = 127.0 * 128.0 - SHIFT * SA - 5.5

_NC_CACHE = {}


def _build_nc():
    from contextlib import ExitStack

    import concourse.bacc as bacc
    import concourse.bass as bass
    import concourse.mybir as mybir
    import concourse.tile as tile

    dt = mybir.dt
    nc = bacc.Bacc(target_bir_lowering=False)

    xT_d = nc.declare_dram_parameter("xT", [128, N], dt.float16, isOutput=False)
    w_d = nc.declare_dram_parameter("w", [128, NI], dt.float16, isOutput=False)
    v_d = nc.declare_dram_parameter(
        "v", [N // 512, 128, 4, 128], dt.bfloat16, isOutput=False
    )
    xres_d = nc.declare_dram_parameter(
        "xres", [NI // 128, 128, 128], dt.float32, isOutput=False
    )
    gam_d = nc.declare_dram_parameter("gam", [1, 1], dt.float32, isOutput=False)
    out_d = nc.declare_dram_parameter(
        "out", [NI // 128, 128, 128], dt.float32, isOutput=True
    )

    NCH = N // 512   # 8 column chunks of xT / row chunks of v
    with tile.TileContext(nc) as tc, ExitStack() as ctx:
        persist = ctx.enter_context(tc.tile_pool(name="persist", bufs=1))

        # warm up the exp table load while DMAs run
        dummy = persist.tile([1, 1], dt.float32)
        nc.vector.memset(dummy[:], 0.0)
        nc.scalar.activation(
            out=dummy[:], in_=dummy[:], func=mybir.ActivationFunctionType.Exp
        )
        # zeroed operand for PE-warmup matmuls
        warm = persist.tile([128, 128], dt.float16)
        nc.vector.memset(warm[:], 0.0)

        # tiny queue-warmers absorb DMA-ring init latency, then DMA issue
        # order mirrors first use: xt0+w0 feed the first energy group, v0
        # the first attn@v; later chunks alternate between the two queues
        qw = persist.tile([1, 4], dt.float16)
        nc.sync.dma_start(out=qw[0:1, 0:2], in_=xT_d[0:1, 0:2])
        nc.gpsimd.dma_start(out=qw[0:1, 2:4], in_=xT_d[0:1, 2:4])

        xt_ch = [
            persist.tile([128, 512], dt.float16, name=f"xt{jc}") for jc in range(NCH)
        ]
        v_ch = [
            persist.tile([128, 4, 132], dt.bfloat16, name=f"v{jc}")
            for jc in range(NCH)
        ]
        w_t = persist.tile([128, NI], dt.float16)

        def ld_xt(eng, jc):
            eng.dma_start(out=xt_ch[jc][:], in_=xT_d[:, jc * 512 : (jc + 1) * 512])

        def ld_v(eng, jc):
            eng.dma_start(out=v_ch[jc][:, :, 0:128], in_=v_d[jc])

        def ld_w(eng, k):
            eng.dma_start(
                out=w_t[:, k * 512 : (k + 1) * 512],
                in_=w_d[:, k * 512 : (k + 1) * 512],
            )

        nc.scalar.dma_start(out=w_t[:, 0:256], in_=w_d[:, 0:256])
        nc.sync.dma_start(out=xt_ch[0][:, 0:256], in_=xT_d[:, 0:256])
        nc.scalar.dma_start(out=xt_ch[0][:, 256:512], in_=xT_d[:, 256:512])
        # interleaved by time of first use (energy g needs xt g; attn@v g
        # needs v g one exp later; w columns 256: only at i-chunk 1+)
        ld_xt(nc.gpsimd, 1)
        ld_v(nc.gpsimd, 0)
        ld_xt(nc.sync, 2)
        ld_v(nc.gpsimd, 1)
        ld_xt(nc.sync, 3)
        ld_v(nc.gpsimd, 2)
        ld_xt(nc.sync, 4)
        ld_v(nc.gpsimd, 3)
        ld_xt(nc.sync, 5)
        ld_v(nc.gpsimd, 4)
        ld_xt(nc.sync, 6)
        ld_v(nc.gpsimd, 5)
        ld_xt(nc.sync, 7)
        ld_v(nc.gpsimd, 6)
        ld_v(nc.sync, 7)
        nc.gpsimd.dma_start(out=w_t[:, 256:512], in_=w_d[:, 256:512])
        ld_w(nc.gpsimd, 1)
        ld_w(nc.sync, 2)
        ld_w(nc.gpsimd, 3)
        gam = persist.tile([128, 1], dt.float32)
        gam_ap = gam_d[:, :]
        nc.gpsimd.dma_start(
            out=gam[:],
            in_=bass.AP(
                tensor=gam_ap.tensor, offset=gam_ap.offset, ap=[[0, 128], [1, 1]]
            ),
        )
        shiftb = persist.tile([128, 1], dt.float32)
        nc.vector.memset(shiftb[:], -SHIFT)
        for jc in range(NCH):
            nc.vector.memset(v_ch[jc][:, :, 128:129], 1.0)

        epool = ctx.enter_context(tc.tile_pool(name="epsum", bufs=3, space="PSUM"))
        opool = ctx.enter_context(tc.tile_pool(name="opsum", bufs=1, space="PSUM"))
        ptpool = ctx.enter_context(tc.tile_pool(name="ptp", bufs=8))
        spool = ctx.enter_context(tc.tile_pool(name="small", bufs=8))
        xrpool = ctx.enter_context(tc.tile_pool(name="xrp", bufs=3))
        outpool = ctx.enter_context(tc.tile_pool(name="outp", bufs=3))

        # ---- PE warmup (also spins the pstate clock up) ----
        wt = opool.tile([128, 512], dt.float32, tag="oa0", name="warmp")
        for r in range(8):
            nc.tensor.matmul(
                wt[:, r * 64 : (r + 1) * 64],
                warm[:],
                warm[:, 0:64],
                start=True,
                stop=True,
            )

        # ---- main loop ----
        ngroups = NJB // G

        def emit_energy(icn, gi):
            et = epool.tile([128, G, IC], dt.float32, tag="et", name=f"et{icn}_{gi}")
            wsl = w_t[:, icn * IC : (icn + 1) * IC]
            for g in range(G):
                jb = gi * G + g
                nc.tensor.matmul(
                    et[:, g, :],
                    xt_ch[jb // 4][:, (jb % 4) * 128 : (jb % 4 + 1) * 128],
                    wsl,
                    start=True,
                    stop=True,
                )
            return et

        def emit_exp(icn, gi):
            et = ets.pop((icn, gi))
            pt = ptpool.tile(
                [128, G, IC], dt.bfloat16, tag="pt", name=f"pt{icn}_{gi}"
            )
            # DVE: Schraudolph exp via affine + saturating u16 convert
            nc.vector.tensor_scalar(
                out=pt[:, :, SPLIT:IC].bitcast(dt.uint16),
                in0=et[:, :, SPLIT:IC],
                scalar1=float(SA),
                scalar2=float(SB),
                op0=mybir.AluOpType.mult,
                op1=mybir.AluOpType.add,
            )
            # ACT: exact exp
            nc.scalar.activation(
                out=pt[:, :, 0:SPLIT],
                in_=et[:, :, 0:SPLIT],
                func=mybir.ActivationFunctionType.Exp,
                bias=shiftb[:],
            )
            return pt

        def emit_attnv(icn, gi, pt):
            oat = oa_by_ic[icn]
            for g in range(G):
                jb = gi * G + g
                for it in range(NIT):
                    # start=True clears has_written for the WHOLE bank, so only
                    # it=0 may set it; it=1's first write lands on cleared bits
                    # and therefore overwrites (= start) without re-clearing.
                    nc.tensor.matmul(
                        oat[:, it, :],
                        pt[:, g, it * 128 : (it + 1) * 128],
                        v_ch[jb // 4][:, jb % 4, 0:129],
                        start=(jb == 0 and it == 0),
                        stop=(jb == NJB - 1),
                        skip_group_check=(jb == 0 and it == 1),
                    )

        def alloc_oa(icn):
            # ping-pong by chunk parity so attn@v of chunk n+1 accumulates
            # while blend of chunk n still reads the other bank
            oa_by_ic[icn] = opool.tile(
                [128, NIT, 129], dt.float32, tag=f"oa{icn % 2}", name=f"oat{icn}"
            )

        def emit_blend(icn):
            oat = oa_by_ic.pop(icn)
            for it in range(NIT):
                ti = icn * NIT + it
                rs = spool.tile([128, 1], dt.float32, tag="rs", name=f"rs{ti}")
                nc.vector.reciprocal(rs[:], oat[:, it, 128:129])
                nc.vector.tensor_scalar(
                    out=rs[:],
                    in0=rs[:],
                    scalar1=gam[:],
                    scalar2=None,
                    op0=mybir.AluOpType.mult,
                )
                xr = xrpool.tile([128, 128], dt.float32, tag="xr", name=f"xr{ti}")
                nc.sync.dma_start(out=xr[:], in_=xres_d[ti])
                ot = outpool.tile([128, 128], dt.float32, tag="ot", name=f"ot{ti}")
                # fused: out = (attn_num * gamma/den) + x_residual
                nc.vector.scalar_tensor_tensor(
                    out=ot[:],
                    in0=oat[:, it, 0:128],
                    scalar=rs[:],
                    in1=xr[:],
                    op0=mybir.AluOpType.mult,
                    op1=mybir.AluOpType.add,
                )
                nc.gpsimd.dma_start(out=out_d[ti], in_=ot[:])

        ets = {}
        oa_by_ic = {}

        # uniform flat schedule over all (i-chunk, group) pairs with 3-group
        # PE lookahead (3 et slots). The lookahead energy is emitted AFTER
        # this group's attn@v so a slot-blocked energy never head-of-line-
        # blocks ready attn@v work on the PE queue. blend(n) is emitted one
        # group into chunk n+1 so its vector ops don't delay the next chunk's
        # exp on the DVE queue.
        flat = [(icn, gi) for icn in range(NICH) for gi in range(ngroups)]
        for k in range(3):
            ets[flat[k]] = emit_energy(*flat[k])
        for fk, (icn, gi) in enumerate(flat):
            pt = emit_exp(icn, gi)
            if gi == 0:
                alloc_oa(icn)
            emit_attnv(icn, gi, pt)
            if fk + 3 < len(flat):
                ets[flat[fk + 3]] = emit_energy(*flat[fk + 3])
            if gi == 1 and icn > 0:
                emit_blend(icn - 1)
        emit_blend(NICH - 1)

    nc.finalize()
    return nc


def get_nc():
    if "nc" not in _NC_CACHE:
        _NC_CACHE["nc"] = _build_nc()
    return _NC_CACHE["nc"]


def make_in_maps(x, Wq, Wk, Wv, gamma):
    import ml_dtypes

    x = np.asarray(x, dtype=np.float32)
    Wq = np.asarray(Wq, dtype=np.float32)
    Wk = np.asarray(Wk, dtype=np.float32)
    Wv = np.asarray(Wv, dtype=np.float32)
    gamma = np.asarray(gamma, dtype=np.float32)

    xf = x.reshape(B, N, C)
    A = Wq @ Wk.T
    gam = gamma.reshape(1, 1)

    in_maps = []
    for c in range(NCORES):
        b, ih = c // 2, c % 2
        xT = xf[b].T  # [128, 4096]
        # rotate the j-order so this core's own i-rows are columns 0:NI
        # (softmax sums over j, so any j-order works as long as v matches)
        xTr = np.ascontiguousarray(np.roll(xT, -ih * NI, axis=1)).astype(np.float16)
        sl = slice(ih * NI, (ih + 1) * NI)
        # w[c, i] = (A @ x_i)_c for this core's i rows (host prep, fp32->fp16)
        w = (A @ xf[b][sl].T).astype(np.float16)
        # v rows follow the same rotated j-order; laid out [jc, p, k, c] to
        # match the [128p, 4k, 128c] SBUF tiles
        v = np.roll(xf[b] @ Wv, -ih * NI, axis=0)
        v = np.ascontiguousarray(
            v.reshape(NCH_V, 4, 128, C).transpose(0, 2, 1, 3)
        ).astype(ml_dtypes.bfloat16)
        in_maps.append(
            {
                "xT": xTr,
                "w": np.ascontiguousarray(w),
                "v": v,
                "xres": np.ascontiguousarray(
                    xf[b][sl].reshape(NI // 128, 128, 128)
                ),
                "gam": gam,
            }
        )
    return in_maps


NCH_V = N // 512


def assemble_out(results):
    outs = [np.asarray(results[c]["out"]).reshape(NI, C) for c in range(NCORES)]
    full = np.stack(
        [np.concatenate([outs[2 * b], outs[2 * b + 1]], axis=0) for b in range(B)]
    )
    return full.reshape(B, Dd, Hh, Ww, C).astype(np.float32)


def kernel(x, Wq, Wk, Wv, gamma):
    from concourse.bass_utils import run_bass_kernel_spmd

    nc = get_nc()
    in_maps = make_in_maps(x, Wq, Wk, Wv, gamma)
    res = run_bass_kernel_spmd(nc, in_maps, core_ids=list(range(NCORES)))
    return assemble_out(res.results)
